# revision 58
# baseline (speedup 1.0000x reference)
"""BiMamba2D (VMamba SS2D) forward on 8 Trainium2 NeuronCores.

Stage 1: core = (direction k, batch b). Full pipeline per direction:
in_proj+conv fused 9-shift matmul -> SiLU -> B/C/delta projections ->
softplus -> selective scan (24 tiles of 8d x 16n on 128 partitions) ->
n-sum via one-hot matmul, y/u written to HBM (y straight from PSUM).

Engine split (hardware ISA limits: scan is DVE-only; GPSIMD cannot touch
PSUM; matmul PSUM outs are <=512 f32 and bank-aligned):
  PE   : front conv matmuls, projections, n-sum, deltaA matmuls (8 tiles)
  ACT  : tanh (SiLU), 2-pass softplus (exp+ln), exp(A*delta), PSUM copies
  DVE  : all 24 scans, most dBu/yp bf16-2x multiplies, silu-stt, w
  Pool : the remaining dBu/yp multiplies (SBUF-only), hstate carries
  DMA  : w broadcasts (24 tiles), deltaA broadcasts (16 tiles), io

Stage 2 = (batch b, L-quarter): host pre-sums the 4 de-permuted direction
partials; device does D*u add, LayerNorm stats, gate, out-projection.

Directions are handled by host-side input transposes/flips (conv weights
transformed accordingly); host de-permutes partials between launches.
"""
import numpy as np

from concourse import bacc, bass, mybir, tile
from concourse.bass_utils import run_bass_kernel_spmd
from concourse.mybir import ActivationFunctionType as AF
from concourse.mybir import AluOpType as ALU

F32 = mybir.dt.float32
F32R = mybir.dt.float32r
BF16 = mybir.dt.bfloat16

B, H, W = 2, 64, 64
L = H * W                 # 4096
C = 96                    # d_model
D = 192                   # d_inner
N = 16                    # d_state
R = 6                     # dt_rank
K = 4
EPS = 1e-5
NT = 24                   # channel tiles of 128 = (8 d) x (16 n)
ROWP = W + 1              # padded row width 65 (zero spacer col kills wraps)
XPAD_LEN = 4356           # 66 rows of 65 + slack; data rows at 66 + h*65
XOFF = 66
SHIFTS = [(dy, dx) for dy in (-1, 0, 1) for dx in (-1, 0, 1)]
DT = [(0, 128), (128, 64)]   # d-dimension partition tiles

SC = 1024                 # super-chunk length
NSC = L // SC
SUB = 1024                # scan/nsum sub-chunk
PE_DELTA_TILES = 8        # tiles 0..k get deltaA via PE matmul; rest via DMA
SCAN_DVE = 5              # scan tiles on DVE; the rest on Pool


# ---------------------------------------------------------------- host side

def _timg(img, k):
    """Transform [..., H, W] so row-major scan == direction-k sequence."""
    if k == 0:
        return img
    if k == 1:
        return np.swapaxes(img, -1, -2)
    if k == 2:
        return img[..., ::-1, ::-1]
    return np.swapaxes(img, -1, -2)[..., ::-1, ::-1]


def host_prep(inputs):
    import ml_dtypes
    x = np.ascontiguousarray(np.asarray(inputs['x'], np.float32))
    in_proj_w = np.asarray(inputs['in_proj_w'], np.float32)
    conv_w = np.asarray(inputs['conv_w'], np.float32)
    conv_b = np.asarray(inputs['conv_b'], np.float32)
    xpw = np.asarray(inputs['x_proj_weight'], np.float32)
    dtw = np.asarray(inputs['dt_projs_weight'], np.float32)
    dtb = np.asarray(inputs['dt_projs_bias'], np.float32)
    A_logs = np.asarray(inputs['A_logs'], np.float32)
    Wi = in_proj_w[:D]

    p = {}
    for k in range(K):
        for b in range(B):
            img = _timg(np.moveaxis(x[b], -1, 0), k)          # [C, H, W]
            xp = np.zeros((C + 1, XPAD_LEN), np.float32)
            rows = xp[:C, XOFF:XOFF + H * ROWP].reshape(C, H, ROWP)
            rows[:, :, :W] = img
            xp[C, :] = 1.0      # bias channel (read by center shift only)
            p[f'xpad_{k}_{b}'] = xp

        kern = _timg(conv_w[:, 0], k)                         # [D, 3, 3]
        Wbig = np.zeros((9, C + 1, D), np.float32)
        for s, (dy, dx) in enumerate(SHIFTS):
            Wbig[s, :C] = (kern[:, dy + 1, dx + 1][:, None] * Wi).T
        Wbig[4, C] = conv_b     # bias via the ones channel, center shift
        # x0.5: silu(x) = (tanh(0.5x)+1) * 0.5x computed from 0.5x in PSUM
        p[f'wbig_{k}'] = np.ascontiguousarray(
            0.5 * Wbig.transpose(1, 0, 2).reshape(C + 1, 9 * D))

        WB = np.zeros((D, 128), np.float32)
        WC = np.zeros((D, 128), np.float32)
        for q in range(128):
            WB[:, q] = xpw[k, R + q % 16, :]
            WC[:, q] = xpw[k, R + N + q % 16, :]
        p[f'wbrep_{k}'] = WB.astype(ml_dtypes.bfloat16)
        p[f'wcrep_{k}'] = WC.astype(ml_dtypes.bfloat16)
        p[f'wdelta_{k}'] = np.ascontiguousarray(
            (dtw[k] @ xpw[k, :R, :]).T).astype(ml_dtypes.bfloat16)  # [D,D] lhsT
        p[f'dtb_{k}'] = dtb[k].reshape(D, 1)
        A = -np.exp(A_logs[k])                                # [D, N]
        af = np.zeros((128, NT), np.float32)
        for t in range(NT):
            af[:, t] = A[8 * t + np.arange(128) // 16, np.arange(128) % 16]
        p[f'aflat_{k}'] = af
        # per-tile one-hot lhsT with A baked in: out[(m,n), l] = A[8t+m, n] *
        # delta[8t+m, l]; contraction = the source d-tile partitions.
        abig = np.zeros((128, PE_DELTA_TILES * 128), np.float32)
        for t in range(PE_DELTA_TILES):
            d0 = (8 * t) % 128
            for m in range(8):
                for n in range(N):
                    abig[d0 + m, t * 128 + 16 * m + n] = A[8 * t + m, n]
        p[f'abig_{k}'] = abig.astype(ml_dtypes.bfloat16)

    # n-sum one-hot stationaries [128, NT*128] bf16
    sn = np.zeros((NT, 128, 128), np.float32)
    for t in range(NT):
        pout = 8 * t + np.arange(128) // 16
        if t >= 16:
            pout -= 128
        sn[t, np.arange(128), pout] = 1.0
    p['snsum'] = sn.transpose(1, 0, 2).reshape(128, NT * 128).astype(
        ml_dtypes.bfloat16)

    # ---- stage 2 prep
    p['dsum'] = np.asarray(inputs['Ds'], np.float32).sum(0).reshape(D, 1)
    p['gamma'] = np.asarray(inputs['ln_gamma'], np.float32).reshape(D, 1)
    p['beta'] = np.asarray(inputs['ln_beta'], np.float32).reshape(D, 1)
    p['ones'] = np.full((D, 1), 1.0, np.float32)
    p['ones_row'] = np.ones((1, 128), np.float32)
    p['wzT'] = np.ascontiguousarray(in_proj_w[D:].T)          # [96, 192]
    p['woutT'] = np.ascontiguousarray(
        np.asarray(inputs['out_proj_w'], np.float32).T).astype(
        ml_dtypes.bfloat16)                                   # [192, 96]
    for b in range(B):
        xt = np.moveaxis(x[b], -1, 0).reshape(C, L)           # [96, L] row-major
        p[f'xT_{b}'] = np.ascontiguousarray(xt)
    return p


# ------------------------------------------------------------- stage 1 build

def build_stage1():
    nc = bacc.Bacc("TRN2", target_bir_lowering=False, debug=False,
                   num_devices=8)
    din = {}
    din['xpad'] = nc.dram_tensor("xpad", [C + 1, XPAD_LEN], F32R,
                                 kind="ExternalInput")
    din['wbig'] = nc.dram_tensor("wbig", [C + 1, 9 * D], F32R,
                                 kind="ExternalInput")
    din['wbrep'] = nc.dram_tensor("wbrep", [D, 128], BF16, kind="ExternalInput")
    din['wcrep'] = nc.dram_tensor("wcrep", [D, 128], BF16, kind="ExternalInput")
    din['wdelta'] = nc.dram_tensor("wdelta", [D, D], BF16, kind="ExternalInput")
    din['dtb'] = nc.dram_tensor("dtb", [D, 1], F32, kind="ExternalInput")
    din['aflat'] = nc.dram_tensor("aflat", [128, NT], F32, kind="ExternalInput")
    din['abig'] = nc.dram_tensor("abig", [128, PE_DELTA_TILES * 128], BF16,
                                 kind="ExternalInput")
    din['snsum'] = nc.dram_tensor("snsum", [128, NT * 128], BF16,
                                  kind="ExternalInput")
    y_out = nc.dram_tensor("y", [D, L], BF16, kind="ExternalOutput")
    u_out = nc.dram_tensor("u", [D, L], BF16, kind="ExternalOutput")

    with tile.TileContext(nc) as tc:
        _stage1_body(tc, nc, din, y_out, u_out)
    nc.compile()
    return nc


def _stage1_body(tc, nc, din, y_out, u_out):
    from contextlib import ExitStack
    ctx = ExitStack()
    with ctx:
        # ---------- persistent pools
        persist = ctx.enter_context(tc.tile_pool(name="persist", bufs=1))

        wbig = persist.tile([C + 1, 9 * D], F32R, tag="wbig", name="wbig")
        nc.sync.dma_start(wbig[:], din['wbig'].ap())
        xpad = persist.tile([C + 1, XPAD_LEN], F32R, tag="xpad", name="xpad")
        for q in range(NSC):
            r0, r1 = (q * SC) // W, ((q + 1) * SC) // W
            b0 = max(0, XOFF + (r0 - 1) * ROWP - 1)
            b1 = min(XPAD_LEN, XOFF + (r1 + 1) * ROWP + 1)
            nc.sync.dma_start(xpad[:, b0:b1], din['xpad'].ap()[:, b0:b1])
        wb_a = persist.tile([128, 128], BF16, tag="wba", name="wba")
        wb_b = persist.tile([64, 128], BF16, tag="wbb", name="wbb")
        nc.sync.dma_start(wb_a[:], din['wbrep'].ap()[0:128, :])
        nc.sync.dma_start(wb_b[:], din['wbrep'].ap()[128:D, :])
        wc_a = persist.tile([128, 128], BF16, tag="wca", name="wca")
        wc_b = persist.tile([64, 128], BF16, tag="wcb", name="wcb")
        nc.sync.dma_start(wc_a[:], din['wcrep'].ap()[0:128, :])
        nc.sync.dma_start(wc_b[:], din['wcrep'].ap()[128:D, :])
        wdel_a = persist.tile([128, D], BF16, tag="wdela", name="wdela")
        wdel_b = persist.tile([64, D], BF16, tag="wdelb", name="wdelb")
        nc.sync.dma_start(wdel_a[:], din['wdelta'].ap()[0:128, :])
        nc.sync.dma_start(wdel_b[:], din['wdelta'].ap()[128:D, :])
        dtb_a = persist.tile([128, 1], F32, tag="dtba", name="dtba")
        dtb_b = persist.tile([64, 1], F32, tag="dtbb", name="dtbb")
        nc.sync.dma_start(dtb_a[:], din['dtb'].ap()[0:128, :])
        nc.sync.dma_start(dtb_b[:], din['dtb'].ap()[128:D, :])
        aflat = persist.tile([128, NT], F32, tag="aflat", name="aflat")
        nc.sync.dma_start(aflat[:], din['aflat'].ap())
        abig = persist.tile([128, PE_DELTA_TILES * 128], BF16, tag="abig",
                            name="abig")
        nc.sync.dma_start(abig[:], din['abig'].ap())
        snsum = persist.tile([128, NT * 128], BF16, tag="snsum", name="snsum")
        nc.sync.dma_start(snsum[:], din['snsum'].ap())

        hstate = persist.tile([128, NT], BF16, tag="hstate", name="hstate")

        # ---------- pools  (PSUM banks: fr 2x1 + pj/da shared 2x1 +
        # ns 2x2 = 8 exactly)
        scpool = ctx.enter_context(tc.tile_pool(name="scpool", bufs=2))
        work = ctx.enter_context(tc.tile_pool(name="work", bufs=3))
        fr_ps = ctx.enter_context(
            tc.tile_pool(name="frps", bufs=2, space="PSUM"))
        da_ps = ctx.enter_context(
            tc.tile_pool(name="daps", bufs=2, space="PSUM"))
        pj_ps = da_ps
        ns_ps = ctx.enter_context(
            tc.tile_pool(name="nsps", bufs=1, space="PSUM"))

        # ---------------- phase A: front (conv+in_proj+SiLU) for one SC
        FS = 512                    # front psum block (rows of 8)
        def front_block(q, ph, ch):
            """One FS-wide block of the conv front: 18 matmuls + tanh +
            silu-stt.  PE runs in emission order, so these are emitted
            interleaved into the previous SC's scan."""
            if ch == 0:
                ph['u_q'] = [
                    scpool.tile([128, SC], BF16, tag="u_a", name="u_a"),
                    scpool.tile([64, SC], BF16, tag="u_b", name="u_b")]
            u_q = ph['u_q']
            l0 = q * SC + ch * FS
            nrow = FS // W
            pfa = fr_ps.tile([128, FS], F32, tag="fpsa", name="fpsa")
            pfb = fr_ps.tile([64, FS], F32, tag="fpsb", name="fpsb")
            views = [pfa[:], pfb[:]]
            for ti, (d0, dl) in enumerate(DT):
                for s, (dy, dx) in enumerate(SHIFTS):
                    off = XOFF + dy * ROWP + dx + (l0 // W) * ROWP
                    rhs = xpad[:][:, off:off + nrow * ROWP]
                    rhs = rhs.rearrange("p (r c) -> p r c", c=ROWP)
                    rhs = rhs[:, :, 0:W]
                    nc.tensor.matmul(
                        views[ti],
                        wbig[:][:, s * D + d0:s * D + d0 + dl],
                        rhs, start=(s == 0), stop=(s == 8))
            th = work.tile([128, 2 * FS], F32, tag="th", name="th", bufs=2)
            thv = [th[:, 0:FS], th[:64, FS:2 * FS]]  # SBUF views, no bank rule
            for ti, (d0, dl) in enumerate(DT):
                # silu(x) = (tanh(0.5x)+1) * 0.5x ; PSUM holds 0.5x
                nc.scalar.activation(thv[ti], views[ti], AF.Tanh)
            eng = (nc.vector, nc.vector)   # gpsimd cannot read PSUM
            for ti, (d0, dl) in enumerate(DT):
                eng[ti].scalar_tensor_tensor(
                    u_q[ti][:, ch * FS:(ch + 1) * FS], thv[ti], 1.0,
                    views[ti], ALU.add, ALU.mult)
            if ch == SC // FS - 1:
                for ti, (d0, dl) in enumerate(DT):
                    nc.sync.dma_start(
                        u_out.ap()[d0:d0 + dl, q * SC:(q + 1) * SC],
                        u_q[ti][:])

        # ---------------- phase B: projections + softplus + w
        def emit_bc(q, u_q, wa, wb, tag):
            out = scpool.tile([128, SC], BF16, tag=tag, name=tag)
            for ch in range(SC // 512):
                pp = pj_ps.tile([128, 512], F32, tag="pda", name="pp")
                csl = slice(ch * 512, (ch + 1) * 512)
                nc.tensor.matmul(pp[:], wa[:], u_q[0][:, csl],
                                 start=True, stop=False)
                nc.tensor.matmul(pp[:], wb[:], u_q[1][:, csl],
                                 start=False, stop=True)
                nc.scalar.copy(out[:, csl], pp[:])
            return out

        def emit_delta(q, u_q):
            """delta = softplus(wdelta @ u + dtb) = ln(1 + e^(pre+dtb)).
            2-pass: Exp from PSUM (512-wide), Ln SC-wide.  Safe: |pre| << 80
            so e^pre never overflows f32.  ACT funcs all in nl-exp table."""
            d_q = [scpool.tile([128, SC], BF16, tag="d_a", name="d_a"),
                   scpool.tile([64, SC], BF16, tag="d_b", name="d_b")]
            en = [work.tile([128, SC], BF16, tag="en_a", name="en_a", bufs=2),
                  work.tile([64, SC], BF16, tag="en_b", name="en_b", bufs=2)]
            for ti, (d0, dl) in enumerate(DT):
                db = dtb_a if ti == 0 else dtb_b
                for ch in range(SC // 512):
                    pp = pj_ps.tile([128, 512], F32, tag="pda", name="ppd")
                    csl = slice(ch * 512, (ch + 1) * 512)
                    nc.tensor.matmul(pp[:dl, :], wdel_a[:][:, d0:d0 + dl],
                                     u_q[0][:, csl], start=True, stop=False)
                    nc.tensor.matmul(pp[:dl, :], wdel_b[:][:, d0:d0 + dl],
                                     u_q[1][:, csl], start=False, stop=True)
                    nc.scalar.activation(en[ti][:, csl], pp[:dl, :],
                                         AF.Exp, bias=db[:, 0:1])
                nc.scalar.activation(d_q[ti][:], en[ti][:], AF.Ln, bias=1.0)
            return d_q

        def emit_w(q, u_q, d_q):
            w_q = [scpool.tile([128, SC], BF16, tag="w_a", name="w_a"),
                   scpool.tile([64, SC], BF16, tag="w_b", name="w_b")]
            for ti in range(2):
                nc.vector.tensor_tensor(w_q[ti][:], d_q[ti][:], u_q[ti][:],
                                        ALU.mult)
            return w_q

        # ---------------- phase C: scan (all 24 tiles are 128 partitions =
        # 8 d-groups x 16 n; only the n-sum output width is 128 vs 64)
        def bcast(ph, t):
            d_q, w_q = ph['d_q'], ph['w_q']
            ti = 0 if t < 16 else 1
            r0 = 8 * t - (0 if t < 16 else 128)
            wr = work.tile([128, SC], BF16, tag="wrep", name=f"wrep{t}",
                           bufs=4)
            nc.sync.dma_start(
                wr[:], w_q[ti][r0:r0 + 8, :].unsqueeze(1)
                .broadcast_to([8, 16, SC]))
            ph['wrep'][t] = wr
            if t >= PE_DELTA_TILES:
                dr = work.tile([128, SC], BF16, tag="drep",
                               name=f"drep{t}", bufs=3)
                nc.sync.dma_start(
                    dr[:], d_q[ti][r0:r0 + 8, :].unsqueeze(1)
                    .broadcast_to([8, 16, SC]))
                ph['drep'][t] = dr

        def phase_steps(q, ph):
            """Thunks for phases A/B of super-chunk q, to be interleaved
            into the previous SC's scan emission (engines run in-order)."""
            steps = [lambda ch=ch: front_block(q, ph, ch)
                     for ch in range(SC // FS)]
            steps.append(lambda: ph.__setitem__(
                'bbc', emit_bc(q, ph['u_q'], wb_a, wb_b, "bbc")))
            steps.append(lambda: ph.__setitem__(
                'cbc', emit_bc(q, ph['u_q'], wc_a, wc_b, "cbc")))
            steps.append(lambda: ph.__setitem__(
                'd_q', emit_delta(q, ph['u_q'])))
            steps.append(lambda: ph.__setitem__(
                'w_q', emit_w(q, ph['u_q'], ph['d_q'])))
            steps.append(lambda: [bcast(ph, t) for t in range(3)])
            return steps

        def emit_scan(q, ph, nxt_steps):
            d_q, bbc, cbc = ph['d_q'], ph['bbc'], ph['cbc']
            wrep, drep = ph['wrep'], ph['drep']
            nh = SC // SUB
            psacc = [ns_ps.tile([128, SUB], F32, tag="psacc",
                                name=f"psacc{q}_{hb}") for hb in range(nh)]
            hsave = {}

            def stage_head(t):
                """exp + dBu + scan for tile t."""
                dA = work.tile([128, SC], F32, tag="dA", name=f"dA{t}",
                               bufs=3)
                if t < PE_DELTA_TILES:
                    # deltaA via PE: lhsT has A baked; contraction = d-tile A
                    for sb4 in range(SC // 512):
                        pda = da_ps.tile([128, 512], F32, tag="pda",
                                         name="pda")
                        ssl = slice(sb4 * 512, (sb4 + 1) * 512)
                        nc.tensor.matmul(
                            pda[:], abig[:][:, t * 128:t * 128 + 128],
                            d_q[0][:, ssl], start=True, stop=True)
                        nc.scalar.activation(dA[:, ssl], pda[:], AF.Exp)
                else:
                    nc.scalar.activation(dA[:], drep.pop(t)[:], AF.Exp,
                                         scale=aflat[:, t:t + 1])
                dBu = work.tile([128, SC], BF16, tag="dBu", name=f"dBu{t}",
                                bufs=3)
                # walrus: scan is DVE-only; Pool (gpsimd) takes every other
                # dBu/yp multiply (SBUF-only operands)
                emul = nc.gpsimd if t % 2 == 0 else nc.vector
                emul.tensor_tensor(dBu[:], wrep.pop(t)[:], bbc[:], ALU.mult)
                h = work.tile([128, SC], BF16, tag="h", name=f"h{t}", bufs=4)
                init = 0.0 if q == 0 else hstate[:, t:t + 1]
                nc.vector.tensor_tensor_scan(h[:], dA[:], dBu[:], init,
                                             ALU.mult, ALU.add)
                if q < NSC - 1:
                    nc.gpsimd.tensor_copy(hstate[:, t:t + 1],
                                          h[:, SC - 1:SC])
                hsave[t] = h

            def stage_tail(t):
                """yp + n-sum (+ y writeback) for tile t."""
                yp = work.tile([128, SC], BF16, tag="yp", name=f"yp{t}",
                               bufs=3)
                emul = nc.gpsimd if t % 3 == 1 else nc.vector
                emul.tensor_tensor(yp[:], hsave.pop(t)[:], cbc[:], ALU.mult)
                dlo = 128 if t < 16 else 64
                for hb in range(nh):
                    for nb in range(SUB // 512):
                        bsl = slice(nb * 512, (nb + 1) * 512)
                        nc.tensor.matmul(
                            psacc[hb][:dlo, bsl],
                            snsum[:][:, t * 128:t * 128 + dlo],
                            yp[:, hb * SUB + nb * 512:
                               hb * SUB + (nb + 1) * 512],
                            start=(t in (0, 16)), stop=(t in (15, 23)))
                    if t == 15 or t == 23:
                        d0 = 0 if t == 15 else 128
                        ysb = work.tile([128, SUB], BF16, tag="ysb",
                                        name=f"ysb{t}_{hb}", bufs=3)
                        nc.scalar.copy(ysb[:dlo, :], psacc[hb][:dlo, :])
                        nc.sync.dma_start(
                            y_out.ap()[d0:d0 + dlo,
                                       q * SC + hb * SUB:
                                       q * SC + (hb + 1) * SUB],
                            ysb[:dlo, :])

            # milestones: spread next-SC phase emission across the scan
            nst = len(nxt_steps) if nxt_steps else 0
            slots = set()
            if nst:
                for i in range(nst):
                    slots.add(3 + (i * (NT - 5)) // nst)
            for t in range(NT):
                if t + 3 < NT:
                    bcast(ph, t + 3)
                stage_head(t)
                if nxt_steps and t in slots:
                    nxt_steps.pop(0)()
                if t >= 1:
                    stage_tail(t - 1)
            stage_tail(NT - 1)
            while nxt_steps:
                nxt_steps.pop(0)()
            return

        # ---------------- schedule: 2 super-chunks; SC q+1's front/proj
        # emission is interleaved into SC q's scan (engines are in-order)
        phs = [dict(wrep={}, drep={}) for _ in range(NSC)]
        for step in phase_steps(0, phs[0]):
            step()
        for q in range(NSC):
            nxt = phase_steps(q + 1, phs[q + 1]) if q + 1 < NSC else None
            emit_scan(q, phs[q], nxt)


# ------------------------------------------------------------- stage 2 build

def build_stage2():
    nc = bacc.Bacc("TRN2", target_bir_lowering=False, debug=False,
                   num_devices=8)
    LQ = L // 4
    din = {}
    din['ysum4'] = nc.dram_tensor("ysum4", [D, LQ], F32R,
                                  kind="ExternalInput")
    din['ubase'] = nc.dram_tensor("ubase", [D, LQ], BF16, kind="ExternalInput")
    din['xT'] = nc.dram_tensor("xT", [C, LQ], F32R, kind="ExternalInput")
    din['dsum'] = nc.dram_tensor("dsum", [D, 1], F32, kind="ExternalInput")
    din['gamma'] = nc.dram_tensor("gamma", [D, 1], F32, kind="ExternalInput")
    din['beta'] = nc.dram_tensor("beta", [D, 1], F32, kind="ExternalInput")
    din['ones'] = nc.dram_tensor("ones", [D, 1], F32R, kind="ExternalInput")
    din['ones_row'] = nc.dram_tensor("ones_row", [1, 128], F32R,
                                     kind="ExternalInput")
    din['wzT'] = nc.dram_tensor("wzT", [C, D], F32R, kind="ExternalInput")
    din['woutT'] = nc.dram_tensor("woutT", [D, C], BF16, kind="ExternalInput")
    o_out = nc.dram_tensor("o", [C, LQ], F32, kind="ExternalOutput")

    with tile.TileContext(nc) as tc:
        _stage2_body(tc, nc, din, o_out, LQ)
    nc.compile()
    return nc


def _stage2_body(tc, nc, din, o_out, LQ):
    dls = (128, 64)
    with tc.tile_pool(name="sb", bufs=1) as sb:
        ys4 = [sb.tile([128, LQ], F32R, tag="y4a", name="y4a"),
               sb.tile([64, LQ], F32R, tag="y4b", name="y4b")]
        nc.sync.dma_start(ys4[0][:], din['ysum4'].ap()[0:128, :])
        nc.sync.dma_start(ys4[1][:], din['ysum4'].ap()[128:D, :])
        ub = [sb.tile([128, LQ], BF16, tag="uba", name="uba"),
              sb.tile([64, LQ], BF16, tag="ubb", name="ubb")]
        nc.sync.dma_start(ub[0][:], din['ubase'].ap()[0:128, :])
        nc.sync.dma_start(ub[1][:], din['ubase'].ap()[128:D, :])
        xT = sb.tile([C, LQ], F32R, tag="xT", name="xT")
        nc.sync.dma_start(xT[:], din['xT'].ap())
        vec = {}
        for nm in ('dsum', 'gamma', 'beta', 'ones'):
            dt_v = F32R if nm == 'ones' else F32
            vec[nm] = (sb.tile([128, 1], dt_v, tag=nm + "a", name=nm + "a"),
                       sb.tile([64, 1], dt_v, tag=nm + "b", name=nm + "b"))
            nc.sync.dma_start(vec[nm][0][:], din[nm].ap()[0:128, :])
            nc.sync.dma_start(vec[nm][1][:], din[nm].ap()[128:D, :])
        ones_row = sb.tile([1, 128], F32R, tag="ones_row", name="ones_row")
        nc.sync.dma_start(ones_row[:], din['ones_row'].ap())
        wzT = sb.tile([C, D], F32R, tag="wzT", name="wzT")
        nc.sync.dma_start(wzT[:], din['wzT'].ap())
        wo = [sb.tile([128, C], BF16, tag="woa", name="woa"),
              sb.tile([64, C], BF16, tag="wob", name="wob")]
        nc.sync.dma_start(wo[0][:], din['woutT'].ap()[0:128, :])
        nc.sync.dma_start(wo[1][:], din['woutT'].ap()[128:D, :])

        # ysum = ysum4 + dsum * u     (stt: (u * dsum) + ysum4)
        ysum = [sb.tile([128, LQ], F32R, tag="ysa", name="ysa"),
                sb.tile([64, LQ], F32R, tag="ysb", name="ysb")]
        sq = [sb.tile([128, LQ], F32R, tag="sqa", name="sqa"),
              sb.tile([64, LQ], F32R, tag="sqb", name="sqb")]
        for ti in range(2):
            nc.vector.scalar_tensor_tensor(
                ysum[ti][:], ub[ti][:], vec['dsum'][ti][:, 0:1],
                ys4[ti][:].bitcast(F32), ALU.mult, ALU.add)
            nc.scalar.square(sq[ti][:], ysum[ti][:])

        # LN stats over channel dim via ones-matmul
        mu = sb.tile([1, LQ], F32, tag="mu", name="mu")
        sd = sb.tile([1, LQ], F32, tag="sd", name="sd")
        rstd = sb.tile([1, LQ], F32R, tag="rstd", name="rstd")
        nmu = sb.tile([1, LQ], F32R, tag="nmu", name="nmu")
        with tc.tile_pool(name="ps1", bufs=1, space="PSUM") as ps1:
            pm = ps1.tile([1, LQ], F32, tag="pm", name="pm")
            pm2 = ps1.tile([1, LQ], F32, tag="pm2", name="pm2")
            for q in range(LQ // 512):
                qsl = slice(q * 512, (q + 1) * 512)
                nc.tensor.matmul(pm[:, qsl], vec['ones'][0][:],
                                 ysum[0][:, qsl], start=True, stop=False)
                nc.tensor.matmul(pm[:, qsl], vec['ones'][1][:],
                                 ysum[1][:, qsl], start=False, stop=True)
                nc.tensor.matmul(pm2[:, qsl], vec['ones'][0][:],
                                 sq[0][:, qsl], start=True, stop=False)
                nc.tensor.matmul(pm2[:, qsl], vec['ones'][1][:],
                                 sq[1][:, qsl], start=False, stop=True)
            nc.scalar.mul(mu[:], pm[:], 1.0 / D)
            # var = pm2/D - mu^2 ;  sd = sqrt(var + eps)
            mu2 = sb.tile([1, LQ], F32, tag="mu2", name="mu2")
            nc.scalar.square(mu2[:], mu[:])
            var = sb.tile([1, LQ], F32, tag="var", name="var")
            nc.vector.scalar_tensor_tensor(var[:], pm2[:], 1.0 / D, mu2[:],
                                           ALU.mult, ALU.subtract)
            nc.vector.tensor_scalar_add(var[:], var[:], EPS)
            nc.scalar.activation(sd[:], var[:], AF.Sqrt)
        with nc.allow_low_precision(reason="f32r rounding for bcast matmul"):
            nc.vector.reciprocal(rstd[:], sd[:])
            nc.vector.tensor_tensor(nmu[:], mu[:], rstd[:].bitcast(F32),
                                    ALU.mult)

        yf = [sb.tile([128, LQ], BF16, tag="yfa", name="yfa"),
              sb.tile([64, LQ], BF16, tag="yfb", name="yfb")]
        with tc.tile_pool(name="ps2", bufs=1, space="PSUM") as ps2:
            # broadcast rstd and mu*rstd across partitions via 1-row matmul
            prs = ps2.tile([128, LQ], F32, tag="prs", name="prs")
            pmu = ps2.tile([128, LQ], F32, tag="pmu", name="pmu")
            pz = [ps2.tile([128, LQ], F32, tag="pza", name="pza"),
                  ps2.tile([64, LQ], F32, tag="pzb", name="pzb")]
            for q in range(LQ // 512):
                qsl = slice(q * 512, (q + 1) * 512)
                nc.tensor.matmul(prs[:, qsl], ones_row[:], rstd[:, qsl],
                                 start=True, stop=True)
                nc.tensor.matmul(pmu[:, qsl], ones_row[:], nmu[:, qsl],
                                 start=True, stop=True)
            for ti, (d0, dl) in enumerate(DT):
                for q in range(LQ // 512):
                    qsl = slice(q * 512, (q + 1) * 512)
                    nc.tensor.matmul(pz[ti][:, qsl],
                                     wzT[:][:, d0:d0 + dl],
                                     xT[:, qsl], start=True, stop=True)

            for ti in range(2):
                dl = dls[ti]
                # t1 = ysum * rstd_bc - (mu*rstd)_bc
                t1 = sb.tile([dl, LQ], F32, tag=f"t1{ti}", name=f"t1{ti}")
                nc.vector.tensor_tensor(t1[:], ysum[ti][:].bitcast(F32),
                                        prs[:dl, :], ALU.mult)
                t2 = sb.tile([dl, LQ], F32, tag=f"t2{ti}", name=f"t2{ti}")
                nc.vector.tensor_tensor(t2[:], t1[:], pmu[:dl, :],
                                        ALU.subtract)
                yn = sb.tile([dl, LQ], BF16, tag=f"yn{ti}", name=f"yn{ti}")
                nc.scalar.activation(yn[:], t2[:], AF.Identity,
                                     bias=vec['beta'][ti][:, 0:1],
                                     scale=vec['gamma'][ti][:, 0:1])
                zt = sb.tile([dl, LQ], BF16, tag=f"z{ti}", name=f"z{ti}")
                nc.scalar.activation(zt[:], pz[ti][:], AF.Silu)
                nc.vector.tensor_tensor(yf[ti][:], yn[:], zt[:], ALU.mult)

        osb = sb.tile([C, LQ], F32, tag="osb", name="osb")
        with tc.tile_pool(name="ps4", bufs=2, space="PSUM") as ps4:
            for q in range(LQ // 512):
                qsl = slice(q * 512, (q + 1) * 512)
                po = ps4.tile([C, 512], F32, tag="po", name="po")
                nc.tensor.matmul(po[:], wo[0][:], yf[0][:, qsl],
                                 start=True, stop=False)
                nc.tensor.matmul(po[:], wo[1][:], yf[1][:, qsl],
                                 start=False, stop=True)
                nc.scalar.copy(osb[:, qsl], po[:])
        nc.sync.dma_start(o_out.ap(), osb[:])


# ---------------------------------------------------------------- execution

_CACHE = {}
LAST_RESULTS = []


def _get_programs():
    if 'nc1' not in _CACHE:
        _CACHE['nc1'] = build_stage1()
        _CACHE['nc2'] = build_stage2()
    return _CACHE['nc1'], _CACHE['nc2']


def kernel(**inputs):
    import os
    trace = bool(os.environ.get('BIMAMBA_TRACE'))
    nc1, nc2 = _get_programs()
    p = host_prep(inputs)

    # stage 1: core = k * 2 + b
    in_maps1 = []
    for core in range(8):
        k, b = core // 2, core % 2
        in_maps1.append({
            'xpad': p[f'xpad_{k}_{b}'],
            'wbig': p[f'wbig_{k}'],
            'wbrep': p[f'wbrep_{k}'],
            'wcrep': p[f'wcrep_{k}'],
            'wdelta': p[f'wdelta_{k}'],
            'dtb': p[f'dtb_{k}'],
            'aflat': p[f'aflat_{k}'],
            'abig': p[f'abig_{k}'],
            'snsum': p['snsum'],
        })
    res1 = run_bass_kernel_spmd(nc1, in_maps1, core_ids=list(range(8)),
                                trace=trace)
    r1 = res1.results

    # host: de-permute partials to row-major, pre-sum directions, slice
    LQ = L // 4
    in_maps2 = []
    ysums = {}
    for b in range(B):
        acc = np.zeros((D, L), np.float32)
        for k in range(4):
            yk = np.asarray(r1[k * 2 + b]['y']).astype(np.float32)
            acc += _timg(yk.reshape(D, H, W), k).reshape(D, L)
        ysums[b] = acc
    for core in range(8):
        b, q = core // 4, core % 4
        ub = np.asarray(r1[0 * 2 + b]['u'])[:, q * LQ:(q + 1) * LQ]
        in_maps2.append({
            'ysum4': np.ascontiguousarray(ysums[b][:, q * LQ:(q + 1) * LQ]),
            'ubase': np.ascontiguousarray(ub),
            'xT': np.ascontiguousarray(p[f'xT_{b}'][:, q * LQ:(q + 1) * LQ]),
            'dsum': p['dsum'],
            'gamma': p['gamma'],
            'beta': p['beta'],
            'ones': p['ones'],
            'ones_row': p['ones_row'],
            'wzT': p['wzT'],
            'woutT': p['woutT'],
        })
    res2 = run_bass_kernel_spmd(nc2, in_maps2, core_ids=list(range(8)),
                                trace=trace)
    r2 = res2.results
    LAST_RESULTS.clear()
    LAST_RESULTS.extend([res1, res2])

    out = np.empty((B, L, C), np.float32)
    for core in range(8):
        b, q = core // 4, core % 4
        out[b, q * LQ:(q + 1) * LQ] = np.asarray(r2[core]['o']).T
    return out.reshape(B, H, W, C)


# revision 79
# speedup vs baseline: 1.1035x; 1.1035x over previous
"""BiMamba2D (VMamba SS2D) forward on 8 Trainium2 NeuronCores.

Stage 1: core = (direction k, batch b). Full pipeline per direction:
in_proj+conv fused 9-shift matmul -> SiLU -> B/C/delta projections ->
softplus -> selective scan (24 tiles of 8d x 16n on 128 partitions) ->
n-sum via one-hot matmul, y/u written to HBM (y straight from PSUM).

Engine split (hardware ISA limits: scan is DVE-only; GPSIMD cannot touch
PSUM; matmul PSUM outs are <=512 f32 and bank-aligned):
  PE   : front conv matmuls, projections, n-sum, deltaA matmuls (8 tiles)
  ACT  : tanh (SiLU), 2-pass softplus (exp+ln), exp(A*delta), PSUM copies
  DVE  : all 24 scans, most dBu/yp bf16-2x multiplies, silu-stt, w
  Pool : the remaining dBu/yp multiplies (SBUF-only), hstate carries
  DMA  : w broadcasts (24 tiles), deltaA broadcasts (16 tiles), io

Stage 2 = (batch b, L-quarter): host pre-sums the 4 de-permuted direction
partials; device does D*u add, LayerNorm stats, gate, out-projection.

Directions are handled by host-side input transposes/flips (conv weights
transformed accordingly); host de-permutes partials between launches.
"""
import numpy as np

from concourse import bacc, bass, mybir, tile
from concourse.bass_utils import run_bass_kernel_spmd
from concourse.mybir import ActivationFunctionType as AF
from concourse.mybir import AluOpType as ALU

F32 = mybir.dt.float32
F32R = mybir.dt.float32r
BF16 = mybir.dt.bfloat16

B, H, W = 2, 64, 64
L = H * W                 # 4096
C = 96                    # d_model
D = 192                   # d_inner
N = 16                    # d_state
R = 6                     # dt_rank
K = 4
EPS = 1e-5
NT = 24                   # channel tiles of 128 = (8 d) x (16 n)
ROWP = W + 1              # padded row width 65 (zero spacer col kills wraps)
XPAD_LEN = 4356           # 66 rows of 65 + slack; data rows at 66 + h*65
XOFF = 66
SHIFTS = [(dy, dx) for dy in (-1, 0, 1) for dx in (-1, 0, 1)]
DT = [(0, 128), (128, 64)]   # d-dimension partition tiles

SC = 1024                 # super-chunk length
NSC = L // SC
SUB = 1024                # scan/nsum sub-chunk
PE_DELTA_TILES = 8        # tiles 0..k get deltaA via PE matmul; rest via DMA
SCAN_DVE = 5              # scan tiles on DVE; the rest on Pool


# ---------------------------------------------------------------- host side

def _timg(img, k):
    """Transform [..., H, W] so row-major scan == direction-k sequence."""
    if k == 0:
        return img
    if k == 1:
        return np.swapaxes(img, -1, -2)
    if k == 2:
        return img[..., ::-1, ::-1]
    return np.swapaxes(img, -1, -2)[..., ::-1, ::-1]


def host_prep(inputs):
    import ml_dtypes
    x = np.ascontiguousarray(np.asarray(inputs['x'], np.float32))
    in_proj_w = np.asarray(inputs['in_proj_w'], np.float32)
    conv_w = np.asarray(inputs['conv_w'], np.float32)
    conv_b = np.asarray(inputs['conv_b'], np.float32)
    xpw = np.asarray(inputs['x_proj_weight'], np.float32)
    dtw = np.asarray(inputs['dt_projs_weight'], np.float32)
    dtb = np.asarray(inputs['dt_projs_bias'], np.float32)
    A_logs = np.asarray(inputs['A_logs'], np.float32)
    Wi = in_proj_w[:D]

    p = {}
    for k in range(K):
        for b in range(B):
            img = _timg(np.moveaxis(x[b], -1, 0), k)          # [C, H, W]
            xp = np.zeros((C + 1, XPAD_LEN), np.float32)
            rows = xp[:C, XOFF:XOFF + H * ROWP].reshape(C, H, ROWP)
            rows[:, :, :W] = img
            xp[C, :] = 1.0      # bias channel (read by center shift only)
            p[f'xpad_{k}_{b}'] = xp

        kern = _timg(conv_w[:, 0], k)                         # [D, 3, 3]
        Wbig = np.zeros((9, C + 1, D), np.float32)
        for s, (dy, dx) in enumerate(SHIFTS):
            Wbig[s, :C] = (kern[:, dy + 1, dx + 1][:, None] * Wi).T
        Wbig[4, C] = conv_b     # bias via the ones channel, center shift
        # x0.5: silu(x) = (tanh(0.5x)+1) * 0.5x computed from 0.5x in PSUM
        p[f'wbig_{k}'] = np.ascontiguousarray(
            0.5 * Wbig.transpose(1, 0, 2).reshape(C + 1, 9 * D))

        WB = np.zeros((D, 128), np.float32)
        WC = np.zeros((D, 128), np.float32)
        for q in range(128):
            WB[:, q] = xpw[k, R + q % 16, :]
            WC[:, q] = xpw[k, R + N + q % 16, :]
        p[f'wbrep_{k}'] = WB.astype(ml_dtypes.bfloat16)
        p[f'wcrep_{k}'] = WC.astype(ml_dtypes.bfloat16)
        p[f'wdelta_{k}'] = np.ascontiguousarray(
            (dtw[k] @ xpw[k, :R, :]).T).astype(ml_dtypes.bfloat16)  # [D,D] lhsT
        p[f'dtb_{k}'] = dtb[k].reshape(D, 1)
        A = -np.exp(A_logs[k])                                # [D, N]
        af = np.zeros((128, NT), np.float32)
        for t in range(NT):
            af[:, t] = A[8 * t + np.arange(128) // 16, np.arange(128) % 16]
        p[f'aflat_{k}'] = af
        # per-tile one-hot lhsT with A baked in: out[(m,n), l] = A[8t+m, n] *
        # delta[8t+m, l]; contraction = the source d-tile partitions.
        abig = np.zeros((128, PE_DELTA_TILES * 128), np.float32)
        for t in range(PE_DELTA_TILES):
            d0 = (8 * t) % 128
            for m in range(8):
                for n in range(N):
                    abig[d0 + m, t * 128 + 16 * m + n] = A[8 * t + m, n]
        p[f'abig_{k}'] = abig.astype(ml_dtypes.bfloat16)

    # n-sum one-hot stationaries [128, NT*128] bf16
    sn = np.zeros((NT, 128, 128), np.float32)
    for t in range(NT):
        pout = 8 * t + np.arange(128) // 16
        if t >= 16:
            pout -= 128
        sn[t, np.arange(128), pout] = 1.0
    p['snsum'] = sn.transpose(1, 0, 2).reshape(128, NT * 128).astype(
        ml_dtypes.bfloat16)

    # ---- stage 2 prep
    p['vecs'] = np.ascontiguousarray(np.stack([
        np.asarray(inputs['Ds'], np.float32).sum(0),
        np.asarray(inputs['ln_gamma'], np.float32),
        np.asarray(inputs['ln_beta'], np.float32)], axis=1))   # [D, 3]
    p['ones'] = np.full((D, 1), 1.0, np.float32)
    p['ones_row'] = np.ones((1, 128), np.float32)
    p['wzT'] = np.ascontiguousarray(in_proj_w[D:].T)          # [96, 192]
    p['woutT'] = np.ascontiguousarray(
        np.asarray(inputs['out_proj_w'], np.float32).T).astype(
        ml_dtypes.bfloat16)                                   # [192, 96]
    for b in range(B):
        xt = np.moveaxis(x[b], -1, 0).reshape(C, L)           # [96, L] row-major
        p[f'xT_{b}'] = np.ascontiguousarray(xt)
    return p


# ------------------------------------------------------------- stage 1 build

def build_stage1():
    nc = bacc.Bacc("TRN2", target_bir_lowering=False, debug=False,
                   num_devices=8)
    din = {}
    din['xpad'] = nc.dram_tensor("xpad", [C + 1, XPAD_LEN], F32R,
                                 kind="ExternalInput")
    din['wbig'] = nc.dram_tensor("wbig", [C + 1, 9 * D], F32R,
                                 kind="ExternalInput")
    din['wbrep'] = nc.dram_tensor("wbrep", [D, 128], BF16, kind="ExternalInput")
    din['wcrep'] = nc.dram_tensor("wcrep", [D, 128], BF16, kind="ExternalInput")
    din['wdelta'] = nc.dram_tensor("wdelta", [D, D], BF16, kind="ExternalInput")
    din['dtb'] = nc.dram_tensor("dtb", [D, 1], F32, kind="ExternalInput")
    din['aflat'] = nc.dram_tensor("aflat", [128, NT], F32, kind="ExternalInput")
    din['abig'] = nc.dram_tensor("abig", [128, PE_DELTA_TILES * 128], BF16,
                                 kind="ExternalInput")
    din['snsum'] = nc.dram_tensor("snsum", [128, NT * 128], BF16,
                                  kind="ExternalInput")
    y_out = nc.dram_tensor("y", [D, L], BF16, kind="ExternalOutput")
    u_out = nc.dram_tensor("u", [D, L], BF16, kind="ExternalOutput")

    with tile.TileContext(nc) as tc:
        _stage1_body(tc, nc, din, y_out, u_out)
    nc.compile()
    return nc


def _stage1_body(tc, nc, din, y_out, u_out):
    from contextlib import ExitStack
    ctx = ExitStack()
    with ctx:
        # ---------- persistent pools
        persist = ctx.enter_context(tc.tile_pool(name="persist", bufs=1))

        wbig = persist.tile([C + 1, 9 * D], F32R, tag="wbig", name="wbig")
        nc.sync.dma_start(wbig[:], din['wbig'].ap())
        xpad = persist.tile([C + 1, XPAD_LEN], F32R, tag="xpad", name="xpad")
        for q in range(NSC):
            r0, r1 = (q * SC) // W, ((q + 1) * SC) // W
            b0 = max(0, XOFF + (r0 - 1) * ROWP - 1)
            b1 = min(XPAD_LEN, XOFF + (r1 + 1) * ROWP + 1)
            nc.sync.dma_start(xpad[:, b0:b1], din['xpad'].ap()[:, b0:b1])
        wb_a = persist.tile([128, 128], BF16, tag="wba", name="wba")
        wb_b = persist.tile([64, 128], BF16, tag="wbb", name="wbb")
        nc.sync.dma_start(wb_a[:], din['wbrep'].ap()[0:128, :])
        nc.sync.dma_start(wb_b[:], din['wbrep'].ap()[128:D, :])
        wc_a = persist.tile([128, 128], BF16, tag="wca", name="wca")
        wc_b = persist.tile([64, 128], BF16, tag="wcb", name="wcb")
        nc.sync.dma_start(wc_a[:], din['wcrep'].ap()[0:128, :])
        nc.sync.dma_start(wc_b[:], din['wcrep'].ap()[128:D, :])
        wdel_a = persist.tile([128, D], BF16, tag="wdela", name="wdela")
        wdel_b = persist.tile([64, D], BF16, tag="wdelb", name="wdelb")
        nc.sync.dma_start(wdel_a[:], din['wdelta'].ap()[0:128, :])
        nc.sync.dma_start(wdel_b[:], din['wdelta'].ap()[128:D, :])
        dtb_a = persist.tile([128, 1], F32, tag="dtba", name="dtba")
        dtb_b = persist.tile([64, 1], F32, tag="dtbb", name="dtbb")
        nc.sync.dma_start(dtb_a[:], din['dtb'].ap()[0:128, :])
        nc.sync.dma_start(dtb_b[:], din['dtb'].ap()[128:D, :])
        aflat = persist.tile([128, NT], F32, tag="aflat", name="aflat")
        nc.sync.dma_start(aflat[:], din['aflat'].ap())
        abig = persist.tile([128, PE_DELTA_TILES * 128], BF16, tag="abig",
                            name="abig")
        nc.sync.dma_start(abig[:], din['abig'].ap())
        snsum = persist.tile([128, NT * 128], BF16, tag="snsum", name="snsum")
        nc.sync.dma_start(snsum[:], din['snsum'].ap())

        hstate = persist.tile([128, NT], BF16, tag="hstate", name="hstate")

        # ---------- pools  (PSUM banks: fr (2+2) + pj/da 2x1 + ns 2 = 8)
        scpool = ctx.enter_context(tc.tile_pool(name="scpool", bufs=2))
        work = ctx.enter_context(tc.tile_pool(name="work", bufs=3))
        fr_ps = ctx.enter_context(
            tc.tile_pool(name="frps", bufs=1, space="PSUM"))
        da_ps = ctx.enter_context(
            tc.tile_pool(name="daps", bufs=2, space="PSUM"))
        pj_ps = da_ps
        ns_ps = ctx.enter_context(
            tc.tile_pool(name="nsps", bufs=1, space="PSUM"))

        # ---------------- phase A: front (conv+in_proj+SiLU) for one SC.
        # Shift-outer loop: each of the 18 stationaries serves the whole SC
        # (one Ldweights per stationary; reloading resets the PE p-state).
        def front_block(q, ph):
            ph['u_q'] = [
                scpool.tile([128, SC], BF16, tag="u_a", name="u_a"),
                scpool.tile([64, SC], BF16, tag="u_b", name="u_b")]
            u_q = ph['u_q']
            pfr = [fr_ps.tile([128, SC], F32, tag="fpsa", name="fpsa"),
                   fr_ps.tile([64, SC], F32, tag="fpsb", name="fpsb")]
            for s, (dy, dx) in enumerate(SHIFTS):
                for ti, (d0, dl) in enumerate(DT):
                    for hf in range(SC // 512):
                        l0 = q * SC + hf * 512
                        off = XOFF + dy * ROWP + dx + (l0 // W) * ROWP
                        rhs = xpad[:][:, off:off + 8 * ROWP]
                        rhs = rhs.rearrange("p (r c) -> p r c", c=ROWP)
                        rhs = rhs[:, :, 0:W]
                        nc.tensor.matmul(
                            pfr[ti][:, hf * 512:(hf + 1) * 512],
                            wbig[:][:, s * D + d0:s * D + d0 + dl],
                            rhs, start=(s == 0), stop=(s == 8))
            th = work.tile([128, 2 * SC], F32, tag="th", name="th", bufs=2)
            thv = [th[:, 0:SC], th[:64, SC:2 * SC]]
            for ti in range(2):
                # silu(x) = (tanh(0.5x)+1) * 0.5x ; PSUM holds 0.5x
                nc.scalar.activation(thv[ti], pfr[ti][:], AF.Tanh)
            for ti in range(2):
                nc.vector.scalar_tensor_tensor(
                    u_q[ti][:], thv[ti], 1.0, pfr[ti][:],
                    ALU.add, ALU.mult)
            for ti, (d0, dl) in enumerate(DT):
                nc.sync.dma_start(
                    u_out.ap()[d0:d0 + dl, q * SC:(q + 1) * SC],
                    u_q[ti][:])

        # ---------------- phase B: projections + softplus + w
        def emit_bc(q, u_q, wa, wb, tag):
            out = scpool.tile([128, SC], BF16, tag=tag, name=tag)
            pps = [pj_ps.tile([128, 512], F32, tag="pda", name=f"pp{c}")
                   for c in range(SC // 512)]
            for pi, wpart in enumerate((wa, wb)):     # stationary-outer
                for ch, pp in enumerate(pps):
                    csl = slice(ch * 512, (ch + 1) * 512)
                    nc.tensor.matmul(pp[:], wpart[:], u_q[pi][:, csl],
                                     start=(pi == 0), stop=(pi == 1))
            for ch, pp in enumerate(pps):
                nc.scalar.copy(out[:, ch * 512:(ch + 1) * 512], pp[:])
            return out

        def emit_delta(q, u_q):
            """delta = softplus(wdelta @ u + dtb) = ln(1 + e^(pre+dtb)).
            2-pass: Exp from PSUM (512-wide), Ln SC-wide.  Safe: |pre| << 80
            so e^pre never overflows f32.  ACT funcs all in nl-exp table."""
            d_q = [scpool.tile([128, SC], BF16, tag="d_a", name="d_a"),
                   scpool.tile([64, SC], BF16, tag="d_b", name="d_b")]
            en = [work.tile([128, SC], BF16, tag="en_a", name="en_a", bufs=2),
                  work.tile([64, SC], BF16, tag="en_b", name="en_b", bufs=2)]
            # all Exp passes first, then both Ln passes back-to-back: each
            # Ln visit costs 2 table loads (natural_log lacks exp), so
            # adjacency halves the thrash
            for ti, (d0, dl) in enumerate(DT):
                db = dtb_a if ti == 0 else dtb_b
                pps = [pj_ps.tile([128, 512], F32, tag="pda", name=f"pd{c}")
                       for c in range(SC // 512)]
                for pi, wpart in enumerate((wdel_a, wdel_b)):
                    for ch, pp in enumerate(pps):
                        csl = slice(ch * 512, (ch + 1) * 512)
                        nc.tensor.matmul(pp[:dl, :],
                                         wpart[:][:, d0:d0 + dl],
                                         u_q[pi][:, csl],
                                         start=(pi == 0), stop=(pi == 1))
                for ch, pp in enumerate(pps):
                    nc.scalar.activation(en[ti][:, ch * 512:(ch + 1) * 512],
                                         pp[:dl, :], AF.Exp, bias=db[:, 0:1])
            for ti in range(2):
                nc.scalar.activation(d_q[ti][:], en[ti][:], AF.Ln, bias=1.0)
            return d_q

        def emit_w(q, u_q, d_q):
            w_q = [scpool.tile([128, SC], BF16, tag="w_a", name="w_a"),
                   scpool.tile([64, SC], BF16, tag="w_b", name="w_b")]
            for ti in range(2):
                nc.vector.tensor_tensor(w_q[ti][:], d_q[ti][:], u_q[ti][:],
                                        ALU.mult)
            return w_q

        # ---------------- phase C: scan (all 24 tiles are 128 partitions =
        # 8 d-groups x 16 n; only the n-sum output width is 128 vs 64)
        def bcast(ph, t):
            d_q, w_q = ph['d_q'], ph['w_q']
            ti = 0 if t < 16 else 1
            r0 = 8 * t - (0 if t < 16 else 128)
            wr = work.tile([128, SC], BF16, tag="wrep", name=f"wrep{t}",
                           bufs=6)
            nc.sync.dma_start(
                wr[:], w_q[ti][r0:r0 + 8, :].unsqueeze(1)
                .broadcast_to([8, 16, SC]))
            ph['wrep'][t] = wr
            if t >= PE_DELTA_TILES:
                dr = work.tile([128, SC], BF16, tag="drep",
                               name=f"drep{t}", bufs=5)
                nc.sync.dma_start(
                    dr[:], d_q[ti][r0:r0 + 8, :].unsqueeze(1)
                    .broadcast_to([8, 16, SC]))
                ph['drep'][t] = dr

        def phase_steps(q, ph):
            """Thunks for phases A/B of super-chunk q, to be interleaved
            into the previous SC's scan emission (engines run in-order)."""
            steps = [lambda: front_block(q, ph)]
            steps.append(lambda: ph.__setitem__(
                'bbc', emit_bc(q, ph['u_q'], wb_a, wb_b, "bbc")))
            steps.append(lambda: ph.__setitem__(
                'cbc', emit_bc(q, ph['u_q'], wc_a, wc_b, "cbc")))
            steps.append(lambda: ph.__setitem__(
                'd_q', emit_delta(q, ph['u_q'])))
            steps.append(lambda: ph.__setitem__(
                'w_q', emit_w(q, ph['u_q'], ph['d_q'])))
            steps.append(lambda: [bcast(ph, t) for t in range(5)])
            return steps

        def emit_scan(q, ph, nxt_steps):
            d_q, bbc, cbc = ph['d_q'], ph['bbc'], ph['cbc']
            wrep, drep = ph['wrep'], ph['drep']
            nh = SC // SUB
            psacc = [ns_ps.tile([128, SUB], F32, tag="psacc",
                                name=f"psacc{q}_{hb}") for hb in range(nh)]
            hsave = {}
            dbusave = {}

            def stage_dbu(t):
                """dBu multiply, one tile ahead of its scan."""
                dBu = work.tile([128, SC], BF16, tag="dBu", name=f"dBu{t}",
                                bufs=4)
                # walrus: scan is DVE-only; Pool (gpsimd) takes every other
                # dBu/yp multiply (SBUF-only operands)
                emul = nc.gpsimd if t % 2 == 0 else nc.vector
                emul.tensor_tensor(dBu[:], wrep.pop(t)[:], bbc[:], ALU.mult)
                dbusave[t] = dBu

            def stage_head(t):
                """exp + scan for tile t."""
                dA = work.tile([128, SC], F32, tag="dA", name=f"dA{t}",
                               bufs=4)
                if t < PE_DELTA_TILES:
                    # deltaA via PE: lhsT has A baked; contraction = d-tile A
                    for sb4 in range(SC // 512):
                        pda = da_ps.tile([128, 512], F32, tag="pda",
                                         name="pda")
                        ssl = slice(sb4 * 512, (sb4 + 1) * 512)
                        nc.tensor.matmul(
                            pda[:], abig[:][:, t * 128:t * 128 + 128],
                            d_q[0][:, ssl], start=True, stop=True)
                        nc.scalar.activation(dA[:, ssl], pda[:], AF.Exp)
                else:
                    nc.scalar.activation(dA[:], drep.pop(t)[:], AF.Exp,
                                         scale=aflat[:, t:t + 1])
                h = work.tile([128, SC], BF16, tag="h", name=f"h{t}", bufs=4)
                init = 0.0 if q == 0 else hstate[:, t:t + 1]
                nc.vector.tensor_tensor_scan(h[:], dA[:], dbusave.pop(t)[:],
                                             init, ALU.mult, ALU.add)
                if q < NSC - 1:
                    nc.gpsimd.tensor_copy(hstate[:, t:t + 1],
                                          h[:, SC - 1:SC])
                hsave[t] = h

            def stage_tail(t):
                """yp + n-sum (+ y writeback) for tile t."""
                yp = work.tile([128, SC], BF16, tag="yp", name=f"yp{t}",
                               bufs=3)
                emul = nc.gpsimd if t % 3 == 1 else nc.vector
                emul.tensor_tensor(yp[:], hsave.pop(t)[:], cbc[:], ALU.mult)
                dlo = 128 if t < 16 else 64
                for hb in range(nh):
                    for nb in range(SUB // 512):
                        bsl = slice(nb * 512, (nb + 1) * 512)
                        nc.tensor.matmul(
                            psacc[hb][:dlo, bsl],
                            snsum[:][:, t * 128:t * 128 + dlo],
                            yp[:, hb * SUB + nb * 512:
                               hb * SUB + (nb + 1) * 512],
                            start=(t in (0, 16)), stop=(t in (15, 23)))
                    if t == 15 or t == 23:
                        d0 = 0 if t == 15 else 128
                        ysb = work.tile([128, SUB], BF16, tag="ysb",
                                        name=f"ysb{t}_{hb}", bufs=3)
                        nc.scalar.copy(ysb[:dlo, :], psacc[hb][:dlo, :])
                        nc.sync.dma_start(
                            y_out.ap()[d0:d0 + dlo,
                                       q * SC + hb * SUB:
                                       q * SC + (hb + 1) * SUB],
                            ysb[:dlo, :])

            # milestones: spread next-SC phase emission across the scan
            nst = len(nxt_steps) if nxt_steps else 0
            slots = set()
            if nst:
                for i in range(nst):
                    slots.add(3 + (i * (NT - 5)) // nst)
            stage_dbu(0)
            for t in range(NT):
                if t + 5 < NT:
                    bcast(ph, t + 5)
                if t + 1 < NT:
                    stage_dbu(t + 1)
                stage_head(t)
                if nxt_steps and t in slots:
                    nxt_steps.pop(0)()
                if t >= 1:
                    stage_tail(t - 1)
            stage_tail(NT - 1)
            while nxt_steps:
                nxt_steps.pop(0)()
            return

        # ---------------- schedule: 2 super-chunks; SC q+1's front/proj
        # emission is interleaved into SC q's scan (engines are in-order)
        phs = [dict(wrep={}, drep={}) for _ in range(NSC)]
        for step in phase_steps(0, phs[0]):
            step()
        for q in range(NSC):
            nxt = phase_steps(q + 1, phs[q + 1]) if q + 1 < NSC else None
            emit_scan(q, phs[q], nxt)


# ------------------------------------------------------------- stage 2 build

def build_stage2():
    nc = bacc.Bacc("TRN2", target_bir_lowering=False, debug=False,
                   num_devices=8)
    LQ = L // 4
    din = {}
    din['ysum4'] = nc.dram_tensor("ysum4", [D, LQ], F32R,
                                  kind="ExternalInput")
    din['ubase'] = nc.dram_tensor("ubase", [D, LQ], BF16, kind="ExternalInput")
    din['xT'] = nc.dram_tensor("xT", [C, LQ], F32R, kind="ExternalInput")
    din['vecs'] = nc.dram_tensor("vecs", [D, 3], F32, kind="ExternalInput")
    din['ones'] = nc.dram_tensor("ones", [D, 1], F32R, kind="ExternalInput")
    din['ones_row'] = nc.dram_tensor("ones_row", [1, 128], F32R,
                                     kind="ExternalInput")
    din['wzT'] = nc.dram_tensor("wzT", [C, D], F32R, kind="ExternalInput")
    din['woutT'] = nc.dram_tensor("woutT", [D, C], BF16, kind="ExternalInput")
    o_out = nc.dram_tensor("o", [C, LQ], F32, kind="ExternalOutput")

    with tile.TileContext(nc) as tc:
        _stage2_body(tc, nc, din, o_out, LQ)
    nc.compile()
    return nc


def _stage2_body(tc, nc, din, o_out, LQ):
    dls = (128, 64)
    with tc.tile_pool(name="sb", bufs=1) as sb:
        # dsum/gamma/beta packed as one [D,3] input: 2 DMAs, not 6;
        # ones stays separate (needs native F32R for the stats matmul)
        vp = (sb.tile([128, 3], F32, tag="vpa", name="vpa"),
              sb.tile([64, 3], F32, tag="vpb", name="vpb"))
        nc.sync.dma_start(vp[0][:], din['vecs'].ap()[0:128, :])
        nc.sync.dma_start(vp[1][:], din['vecs'].ap()[128:D, :])
        vec = {}
        for vi, nm in enumerate(('dsum', 'gamma', 'beta')):
            vec[nm] = (vp[0][:, vi:vi + 1], vp[1][:, vi:vi + 1])
        vec['ones'] = (sb.tile([128, 1], F32R, tag="onesa", name="onesa"),
                       sb.tile([64, 1], F32R, tag="onesb", name="onesb"))
        nc.sync.dma_start(vec['ones'][0][:], din['ones'].ap()[0:128, :])
        nc.sync.dma_start(vec['ones'][1][:], din['ones'].ap()[128:D, :])
        ys4 = [sb.tile([128, LQ], F32R, tag="y4a", name="y4a"),
               sb.tile([64, LQ], F32R, tag="y4b", name="y4b")]
        nc.sync.dma_start(ys4[0][:], din['ysum4'].ap()[0:128, :])
        nc.sync.dma_start(ys4[1][:], din['ysum4'].ap()[128:D, :])
        ub = [sb.tile([128, LQ], BF16, tag="uba", name="uba"),
              sb.tile([64, LQ], BF16, tag="ubb", name="ubb")]
        nc.sync.dma_start(ub[0][:], din['ubase'].ap()[0:128, :])
        nc.sync.dma_start(ub[1][:], din['ubase'].ap()[128:D, :])
        xT = sb.tile([C, LQ], F32R, tag="xT", name="xT")
        nc.sync.dma_start(xT[:], din['xT'].ap())
        ones_row = sb.tile([1, 128], F32R, tag="ones_row", name="ones_row")
        nc.sync.dma_start(ones_row[:], din['ones_row'].ap())
        wzT = sb.tile([C, D], F32R, tag="wzT", name="wzT")
        nc.sync.dma_start(wzT[:], din['wzT'].ap())
        wo = [sb.tile([128, C], BF16, tag="woa", name="woa"),
              sb.tile([64, C], BF16, tag="wob", name="wob")]
        nc.sync.dma_start(wo[0][:], din['woutT'].ap()[0:128, :])
        nc.sync.dma_start(wo[1][:], din['woutT'].ap()[128:D, :])

        # ysum = ysum4 + dsum * u     (stt: (u * dsum) + ysum4)
        ysum = [sb.tile([128, LQ], F32R, tag="ysa", name="ysa"),
                sb.tile([64, LQ], F32R, tag="ysb", name="ysb")]
        sq = [sb.tile([128, LQ], F32R, tag="sqa", name="sqa"),
              sb.tile([64, LQ], F32R, tag="sqb", name="sqb")]
        for ti in range(2):
            nc.vector.scalar_tensor_tensor(
                ysum[ti][:], ub[ti][:], vec['dsum'][ti][:, 0:1],
                ys4[ti][:].bitcast(F32), ALU.mult, ALU.add)
            nc.scalar.square(sq[ti][:], ysum[ti][:])

        # LN stats over channel dim via ones-matmul
        mu = sb.tile([1, LQ], F32, tag="mu", name="mu")
        sd = sb.tile([1, LQ], F32, tag="sd", name="sd")
        rstd = sb.tile([1, LQ], F32R, tag="rstd", name="rstd")
        nmu = sb.tile([1, LQ], F32R, tag="nmu", name="nmu")
        with tc.tile_pool(name="ps1", bufs=1, space="PSUM") as ps1:
            pm = ps1.tile([1, LQ], F32, tag="pm", name="pm")
            pm2 = ps1.tile([1, LQ], F32, tag="pm2", name="pm2")
            for q in range(LQ // 512):
                qsl = slice(q * 512, (q + 1) * 512)
                nc.tensor.matmul(pm[:, qsl], vec['ones'][0][:],
                                 ysum[0][:, qsl], start=True, stop=False)
                nc.tensor.matmul(pm[:, qsl], vec['ones'][1][:],
                                 ysum[1][:, qsl], start=False, stop=True)
                nc.tensor.matmul(pm2[:, qsl], vec['ones'][0][:],
                                 sq[0][:, qsl], start=True, stop=False)
                nc.tensor.matmul(pm2[:, qsl], vec['ones'][1][:],
                                 sq[1][:, qsl], start=False, stop=True)
            nc.scalar.mul(mu[:], pm[:], 1.0 / D)
            # var = pm2/D - mu^2 ;  sd = sqrt(var + eps)
            mu2 = sb.tile([1, LQ], F32, tag="mu2", name="mu2")
            nc.scalar.square(mu2[:], mu[:])
            var = sb.tile([1, LQ], F32, tag="var", name="var")
            nc.vector.scalar_tensor_tensor(var[:], pm2[:], 1.0 / D, mu2[:],
                                           ALU.mult, ALU.subtract)
            nc.vector.tensor_scalar_add(var[:], var[:], EPS)
            nc.scalar.activation(sd[:], var[:], AF.Sqrt)
        with nc.allow_low_precision(reason="f32r rounding for bcast matmul"):
            nc.vector.reciprocal(rstd[:], sd[:])
            nc.vector.tensor_tensor(nmu[:], mu[:], rstd[:].bitcast(F32),
                                    ALU.mult)

        yf = [sb.tile([128, LQ], BF16, tag="yfa", name="yfa"),
              sb.tile([64, LQ], BF16, tag="yfb", name="yfb")]
        with tc.tile_pool(name="ps2", bufs=1, space="PSUM") as ps2:
            # broadcast rstd and mu*rstd across partitions via 1-row matmul
            prs = ps2.tile([128, LQ], F32, tag="prs", name="prs")
            pmu = ps2.tile([128, LQ], F32, tag="pmu", name="pmu")
            pz = [ps2.tile([128, LQ], F32, tag="pza", name="pza"),
                  ps2.tile([64, LQ], F32, tag="pzb", name="pzb")]
            for q in range(LQ // 512):
                qsl = slice(q * 512, (q + 1) * 512)
                nc.tensor.matmul(prs[:, qsl], ones_row[:], rstd[:, qsl],
                                 start=True, stop=True)
                nc.tensor.matmul(pmu[:, qsl], ones_row[:], nmu[:, qsl],
                                 start=True, stop=True)
            for ti, (d0, dl) in enumerate(DT):
                for q in range(LQ // 512):
                    qsl = slice(q * 512, (q + 1) * 512)
                    nc.tensor.matmul(pz[ti][:, qsl],
                                     wzT[:][:, d0:d0 + dl],
                                     xT[:, qsl], start=True, stop=True)

            for ti in range(2):
                dl = dls[ti]
                # t1 = ysum * rstd_bc - (mu*rstd)_bc
                t1 = sb.tile([dl, LQ], F32, tag=f"t1{ti}", name=f"t1{ti}")
                nc.vector.tensor_tensor(t1[:], ysum[ti][:].bitcast(F32),
                                        prs[:dl, :], ALU.mult)
                t2 = sb.tile([dl, LQ], F32, tag=f"t2{ti}", name=f"t2{ti}")
                nc.vector.tensor_tensor(t2[:], t1[:], pmu[:dl, :],
                                        ALU.subtract)
                yn = sb.tile([dl, LQ], BF16, tag=f"yn{ti}", name=f"yn{ti}")
                nc.scalar.activation(yn[:], t2[:], AF.Identity,
                                     bias=vec['beta'][ti][:, 0:1],
                                     scale=vec['gamma'][ti][:, 0:1])
                zt = sb.tile([dl, LQ], BF16, tag=f"z{ti}", name=f"z{ti}")
                nc.scalar.activation(zt[:], pz[ti][:], AF.Silu)
                nc.vector.tensor_tensor(yf[ti][:], yn[:], zt[:], ALU.mult)

        osb = sb.tile([C, LQ], F32, tag="osb", name="osb")
        with tc.tile_pool(name="ps4", bufs=2, space="PSUM") as ps4:
            for q in range(LQ // 512):
                qsl = slice(q * 512, (q + 1) * 512)
                po = ps4.tile([C, 512], F32, tag="po", name="po")
                nc.tensor.matmul(po[:], wo[0][:], yf[0][:, qsl],
                                 start=True, stop=False)
                nc.tensor.matmul(po[:], wo[1][:], yf[1][:, qsl],
                                 start=False, stop=True)
                nc.scalar.copy(osb[:, qsl], po[:])
        nc.sync.dma_start(o_out.ap(), osb[:])


# ---------------------------------------------------------------- execution

_CACHE = {}
LAST_RESULTS = []


def _get_programs():
    if 'nc1' not in _CACHE:
        _CACHE['nc1'] = build_stage1()
        _CACHE['nc2'] = build_stage2()
    return _CACHE['nc1'], _CACHE['nc2']


def kernel(**inputs):
    import os
    trace = bool(os.environ.get('BIMAMBA_TRACE'))
    nc1, nc2 = _get_programs()
    p = host_prep(inputs)

    # stage 1: core = k * 2 + b
    in_maps1 = []
    for core in range(8):
        k, b = core // 2, core % 2
        in_maps1.append({
            'xpad': p[f'xpad_{k}_{b}'],
            'wbig': p[f'wbig_{k}'],
            'wbrep': p[f'wbrep_{k}'],
            'wcrep': p[f'wcrep_{k}'],
            'wdelta': p[f'wdelta_{k}'],
            'dtb': p[f'dtb_{k}'],
            'aflat': p[f'aflat_{k}'],
            'abig': p[f'abig_{k}'],
            'snsum': p['snsum'],
        })
    res1 = run_bass_kernel_spmd(nc1, in_maps1, core_ids=list(range(8)),
                                trace=trace)
    r1 = res1.results

    # host: de-permute partials to row-major, pre-sum directions, slice
    LQ = L // 4
    in_maps2 = []
    ysums = {}
    for b in range(B):
        acc = np.zeros((D, L), np.float32)
        for k in range(4):
            yk = np.asarray(r1[k * 2 + b]['y']).astype(np.float32)
            acc += _timg(yk.reshape(D, H, W), k).reshape(D, L)
        ysums[b] = acc
    for core in range(8):
        b, q = core // 4, core % 4
        ub = np.asarray(r1[0 * 2 + b]['u'])[:, q * LQ:(q + 1) * LQ]
        in_maps2.append({
            'ysum4': np.ascontiguousarray(ysums[b][:, q * LQ:(q + 1) * LQ]),
            'ubase': np.ascontiguousarray(ub),
            'xT': np.ascontiguousarray(p[f'xT_{b}'][:, q * LQ:(q + 1) * LQ]),
            'vecs': p['vecs'],
            'ones': p['ones'],
            'ones_row': p['ones_row'],
            'wzT': p['wzT'],
            'woutT': p['woutT'],
        })
    res2 = run_bass_kernel_spmd(nc2, in_maps2, core_ids=list(range(8)),
                                trace=trace)
    r2 = res2.results
    LAST_RESULTS.clear()
    LAST_RESULTS.extend([res1, res2])

    out = np.empty((B, L, C), np.float32)
    for core in range(8):
        b, q = core // 4, core % 4
        out[b, q * LQ:(q + 1) * LQ] = np.asarray(r2[core]['o']).T
    return out.reshape(B, H, W, C)
